# revision 41
# baseline (speedup 1.0000x reference)
"""Trainium2 Bass kernel: RoPE causal attention block (B=2, S=2048, D=1024, H=16).

Sharding: (batch, head-group) across 8 cores — core c handles batch c//4 and
heads (c%4)*4 .. +4 (tensor-parallel qkv/o shards). Each core computes a
partial output projection over its 256 channels; host sums the 4 partials per
batch (the unshard step) and transposes back.

Device-side layout notes:
 - x is fed transposed (d on partitions) so it serves as both lhsT (v proj)
   and rhs (q/k proj) without on-chip transposes.
 - q/k are produced transposed [d_head, S] so scores come out as
   scores^T [k_pos, q_pos]; softmax denominators come from an extra ones
   column appended to v (attn@v then yields z rows 0..63 and den at row 64).
 - exp/normalize: exp on ACT straight out of PSUM, denominators reciprocal'd
   on DVE (1-instr approx, ~51 ULP), broadcast across partitions on GpSimd,
   multiplied during the PSUM->SBUF move on DVE.
All matmuls run in bf16 (1 cyc/row on PE vs 4 for f32).
"""

import sys

sys.path.insert(0, "/opt/trn_rl_repo")

import numpy as np
import ml_dtypes

import concourse.bass as bass  # noqa: F401
import concourse.tile as tile
from concourse import bacc, mybir
from concourse.bass_utils import run_bass_kernel_spmd

BF16 = mybir.dt.bfloat16
F32 = mybir.dt.float32

B, S, D = 2, 2048, 1024
H, DH = 16, 64
NCORE = 8
HPC = 4          # heads per core
KB = D // 128    # 8 k-blocks over the model dim
QC = 512         # q-chunk width
NQC = S // QC    # 4 q-chunks
NKT = S // 128   # 16 k-tiles
ROPE_BASE = 10000.0

_cache = {}


def _build():
    nc = bacc.Bacc("TRN2", target_bir_lowering=False, debug=False, num_devices=NCORE)

    x_d = nc.declare_dram_parameter("x", [128, KB, S], BF16, isOutput=False)
    wqk_d = nc.declare_dram_parameter("wqk", [128, KB, 4, 128], BF16, isOutput=False)
    wv_d = nc.declare_dram_parameter("wv", [128, KB, 256], BF16, isOutput=False)
    wo_d = nc.declare_dram_parameter("wo", [128, 2, 1024], BF16, isOutput=False)
    cos_d = nc.declare_dram_parameter("cos", [128, S], BF16, isOutput=False)
    sin_d = nc.declare_dram_parameter("sin", [128, S], BF16, isOutput=False)
    mask_d = nc.declare_dram_parameter("mask", [128, 2, 128], F32, isOutput=False)
    out_d = nc.declare_dram_parameter("out", [D, S], BF16, isOutput=True)

    Exp = mybir.ActivationFunctionType.Exp

    with tile.TileContext(nc) as tc:
        with (
            tc.tile_pool(name="const", bufs=1) as cp,
            tc.tile_pool(name="ppool", bufs=2, space="PSUM") as ppool,
            tc.tile_pool(name="spool", bufs=2, space="PSUM") as spool,
            tc.tile_pool(name="zpool", bufs=2, space="PSUM") as zpool,
            tc.tile_pool(name="epool", bufs=18) as epool,
            tc.tile_pool(name="work", bufs=2) as work,
            tc.tile_pool(name="nrm", bufs=2) as nrm,
            tc.tile_pool(name="ob", bufs=3) as obp,
        ):
            # ---- load constants ----
            wv = cp.tile([128, KB, 256], BF16, tag="wv")
            nc.scalar.dma_start(wv[:], wv_d[:])
            x_sb = cp.tile([128, KB, S], BF16, tag="x")
            for half in range(2):
                for kb in range(KB):
                    nc.sync.dma_start(
                        x_sb[:, kb, half * 1024:(half + 1) * 1024],
                        x_d[:, kb, half * 1024:(half + 1) * 1024],
                    )
            wqk = cp.tile([128, KB, 4, 128], BF16, tag="wqk")
            nc.sync.dma_start(wqk[:], wqk_d[:])
            wo = cp.tile([128, 2, 1024], BF16, tag="wo")
            nc.sync.dma_start(wo[:], wo_d[:])
            cos_sb = cp.tile([128, S], BF16, tag="cos")
            nc.sync.dma_start(cos_sb[:], cos_d[:])
            sin_sb = cp.tile([128, S], BF16, tag="sin")
            nc.sync.dma_start(sin_sb[:], sin_d[:])
            mask_sb = cp.tile([128, 2, 128], F32, tag="mask")
            nc.scalar.dma_start(mask_sb[:], mask_d[:])

            v_sb = cp.tile([128, NKT, HPC, 65], BF16, tag="v")
            nc.gpsimd.memset(v_sb[:, :, :, 64:65], 1.0)

            # warm the ACT exp table while DMAs stream in
            warm = work.tile([1, 8], F32, tag="warm")
            nc.vector.memset(warm[:], 0.0)
            nc.scalar.activation(warm[:], warm[:], Exp, scale=1.0)

            # ---- v projection: v[s, c] (s on partitions) ----
            for t in range(NKT):
                ps = ppool.tile([128, 512], F32, tag="proj")
                for kb in range(KB):
                    nc.tensor.matmul(
                        ps[:, 0:256],
                        x_sb[:, kb, t * 128:(t + 1) * 128],
                        wv[:, kb, :],
                        start=(kb == 0),
                        stop=(kb == KB - 1),
                    )
                nc.vector.tensor_copy(
                    v_sb[:, t, :, 0:64],
                    ps[:, 0:256].rearrange("p (h d) -> p h d", h=HPC),
                )

            # ---- q/k projection, transposed: [128 = 2 heads x 64, S] ----
            # mt 0,1 = q pairs; mt 2,3 = k pairs. Projection + RoPE run in
            # 1024-wide sequence halves; attention on the first half is
            # emitted before the second half projects, so ACT/DVE attention
            # work overlaps PE projection work.
            T = [cp.tile([128, S], BF16, tag=f"T{mt}", name=f"T{mt}") for mt in range(4)]
            Tpre = [cp.tile([128, S], BF16, tag=f"Tpre{mt}", name=f"Tpre{mt}") for mt in range(4)]
            zsb = [cp.tile([128, S], BF16, tag=f"zsb{p}", name=f"zsb{p}") for p in range(2)]

            def emit_qk_proj_half(nn, mts=(0, 2, 1, 3)):
                lo = nn * 1024
                for mt in mts:
                    for n2 in range(2):
                        n = nn * 2 + n2
                        ps = ppool.tile([128, 512], F32, tag="proj", name=f"qk{mt}_{n}")
                        for kb in range(KB):
                            nc.tensor.matmul(
                                ps[:],
                                wqk[:, kb, mt, :],
                                x_sb[:, kb, n * 512:(n + 1) * 512],
                                start=(kb == 0),
                                stop=(kb == KB - 1),
                            )
                        # Tpre copies stay off DVE: during the interleaved
                        # second half, DVE latency gates zt recycling (norm)
                        nc.scalar.copy(Tpre[mt][:, n * 512:(n + 1) * 512], ps[:])
                    # RoPE on this half (partition-dim rotate-half via DMAs)
                    sw = work.tile([128, 1024], BF16, tag="sw")
                    for g in range(4):
                        src = (g + 1) if g % 2 == 0 else (g - 1)
                        nc.scalar.dma_start(
                            sw[g * 32:(g + 1) * 32, :],
                            Tpre[mt][src * 32:(src + 1) * 32, lo:lo + 1024],
                        )
                    t1 = work.tile([128, 1024], BF16, tag="t1")
                    nc.vector.tensor_mul(t1[:], Tpre[mt][:, lo:lo + 1024], cos_sb[:, lo:lo + 1024])
                    t2 = work.tile([128, 1024], BF16, tag="t2")
                    nc.vector.tensor_mul(t2[:], sw[:], sin_sb[:, lo:lo + 1024])
                    nc.vector.tensor_add(T[mt][:, lo:lo + 1024], t1[:], t2[:])

            # ---- attention (scores^T, exp, attn@v with den row) ----
            emit_qk_proj_half(0)
            # second-half projections interleave at pair granularity: one
            # ~7us PE burst per attention group keeps ACT's 2-deep score
            # pipeline from draining during a single long projection stretch
            proj1_at = {(1, 0): (0,), (1, 1): (2,), (2, 0): (1,), (2, 1): (3,)}
            for j in range(NQC):
                for pair in range(2):
                    if (j, pair) in proj1_at:
                        emit_qk_proj_half(1, mts=proj1_at[(j, pair)])
                    kmax = 4 * (j + 1)
                    iorder = list(range(kmax))
                    exs = {}
                    for i in iorder:
                        qlo = max(0, 128 * i - QC * j)
                        sc = spool.tile([128, 2, QC], F32, tag="sc")
                        for hh in range(2):
                            nc.tensor.matmul(
                                sc[:, hh, qlo:QC],
                                T[2 + pair][hh * 64:(hh + 1) * 64, 128 * i:128 * (i + 1)],
                                T[pair][hh * 64:(hh + 1) * 64, QC * j + qlo:QC * (j + 1)],
                                start=True,
                                stop=True,
                            )
                        if 128 * i >= QC * j:
                            nc.vector.tensor_add(
                                sc[:, :, qlo:qlo + 128],
                                sc[:, :, qlo:qlo + 128],
                                mask_sb[:],
                            )
                        ex = epool.tile([128, 2, QC], BF16, tag="ex", name=f"ex{i}")
                        nc.scalar.activation(
                            ex[:, :, qlo:], sc[:, :, qlo:], Exp, scale=0.125
                        )
                        exs[i] = (ex, qlo)
                    for hh in range(2):
                        zt = zpool.tile([128, QC], F32, tag="zt")
                        for n_i, i in enumerate(iorder):
                            ex, qlo = exs[i]
                            nc.tensor.matmul(
                                zt[0:65, qlo:],
                                v_sb[:, i, pair * 2 + hh, :],
                                ex[:, hh, qlo:],
                                start=(n_i == 0),
                                stop=(n_i == kmax - 1),
                            )
                        # normalize: z / den, den = row 64 of zt
                        # (custom-DVE recip reads PSUM wrong at partition 64 -> copy first)
                        den = nrm.tile([1, QC], F32, tag="den")
                        nc.vector.tensor_copy(den[:], zt[64:65, :])
                        rec = nrm.tile([1, QC], F32, tag="rec")
                        nc.vector.reciprocal_approx_fast(rec[:], den[:])
                        bc = nrm.tile([64, QC], F32, tag="bc")
                        nc.gpsimd.partition_broadcast(bc[:], rec[:])
                        nc.vector.tensor_mul(
                            zsb[pair][hh * 64:(hh + 1) * 64, QC * j:QC * (j + 1)],
                            zt[0:64, :],
                            bc[:],
                        )
                # ---- output projection, pipelined one q-chunk behind ----
                for jj in ([j - 1] if j > 0 else []) + ([j] if j == NQC - 1 else []):
                    _emit_outproj(nc, ppool, obp, wo, zsb, out_d, jj)

    nc.compile()
    return nc


def _emit_outproj(nc, ppool, obp, wo, zsb, out_d, j, pairs=(0, 1), pos=None, done=True):
    for m in range(8):
        po = ppool.tile([128, 512], F32, tag="proj", name=f"po{j}_{m}") if pos is None else pos[m]
        for p in pairs:
            nc.tensor.matmul(
                po[:],
                wo[:, p, m * 128:(m + 1) * 128],
                zsb[p][:, QC * j:QC * (j + 1)],
                start=(p == pairs[0] and pos is None),
                stop=(p == pairs[-1] and done),
            )
        if done:
            # ACT, not DVE: these casts otherwise queue ahead of the next
            # group's normalize ops on DVE, which gate zt recycling
            ob = obp.tile([128, 512], BF16, tag="ob", name=f"ob{j}_{m}")
            nc.scalar.copy(ob[:], po[:])
            nc.sync.dma_start(out_d[m * 128:(m + 1) * 128, QC * j:QC * (j + 1)], ob[:])


def _rope_tables():
    inv_freq = 1.0 / (ROPE_BASE ** (np.arange(0, DH, 2, dtype=np.float32) / DH))
    t = np.arange(S, dtype=np.float32)
    freqs = np.outer(t, inv_freq)            # [S, 32]
    cosT = np.cos(freqs).T                   # [32, S]
    sinT = np.sin(freqs).T
    cos128 = np.concatenate([cosT, cosT, cosT, cosT], axis=0)
    sin128 = np.concatenate([-sinT, sinT, -sinT, sinT], axis=0)
    return cos128.astype(ml_dtypes.bfloat16), sin128.astype(ml_dtypes.bfloat16)


def _prep_in_maps(x, w_qkv, w_o):
    cos128, sin128 = _rope_tables()
    kp, qc = np.meshgrid(np.arange(128), np.arange(128), indexing="ij")
    mask1 = np.where(kp <= qc, 0.0, -1e9).astype(np.float32)         # [128k, 128q]
    mask = np.ascontiguousarray(np.stack([mask1, mask1], axis=1))    # [128, 2, 128]

    in_maps = []
    for c in range(NCORE):
        b, hb = c // 4, (c % 4) * HPC
        xb = np.ascontiguousarray(x[b].T)                        # [D, S]
        x_sb = xb.reshape(KB, 128, S).transpose(1, 0, 2)         # [128, KB, S]

        wqk = np.empty((128, KB, 4, 128), np.float32)
        for pair in range(2):
            qrows = w_qkv[(hb + 2 * pair) * DH:(hb + 2 * pair + 2) * DH, :]   # [128, D]
            krows = w_qkv[D + (hb + 2 * pair) * DH:D + (hb + 2 * pair + 2) * DH, :]
            wqk[:, :, pair, :] = qrows.T.reshape(KB, 128, 128).transpose(1, 0, 2)
            wqk[:, :, 2 + pair, :] = krows.T.reshape(KB, 128, 128).transpose(1, 0, 2)

        vrows = w_qkv[2 * D + hb * DH:2 * D + (hb + HPC) * DH, :]             # [256, D]
        wv = vrows.T.reshape(KB, 128, 256).transpose(1, 0, 2)                 # [128, KB, 256]

        wo_blk = w_o[:, hb * DH:hb * DH + 256]                                # [1024, 256]
        wo = wo_blk.T.reshape(2, 128, 1024).transpose(1, 0, 2)                # [128, 2, 1024]

        in_maps.append({
            "x": x_sb.astype(ml_dtypes.bfloat16),
            "wqk": wqk.astype(ml_dtypes.bfloat16),
            "wv": wv.astype(ml_dtypes.bfloat16),
            "wo": wo.astype(ml_dtypes.bfloat16),
            "cos": cos128,
            "sin": sin128,
            "mask": mask,
        })
    return in_maps


def get_nc():
    if "nc" not in _cache:
        _cache["nc"] = _build()
    return _cache["nc"]


def run(x, w_qkv, w_o, **runkw):
    nc = get_nc()
    in_maps = _prep_in_maps(np.asarray(x), np.asarray(w_qkv), np.asarray(w_o))
    res = run_bass_kernel_spmd(nc, in_maps, core_ids=list(range(NCORE)), **runkw)
    out = np.zeros((B, S, D), np.float32)
    for c in range(NCORE):
        out[c // 4] += res.results[c]["out"].astype(np.float32).T
    return out, res


def kernel(x, w_qkv, w_o):
    out, _ = run(x, w_qkv, w_o)
    return out


# revision 42
# speedup vs baseline: 1.0310x; 1.0310x over previous
"""Trainium2 Bass kernel: RoPE causal attention block (B=2, S=2048, D=1024, H=16).

Sharding: (batch, head-group) across 8 cores — core c handles batch c//4 and
heads (c%4)*4 .. +4 (tensor-parallel qkv/o shards). Each core computes a
partial output projection over its 256 channels; host sums the 4 partials per
batch (the unshard step) and transposes back.

Device-side layout notes:
 - x is fed transposed (d on partitions) so it serves as both lhsT (v proj)
   and rhs (q/k proj) without on-chip transposes.
 - q/k are produced transposed [d_head, S] so scores come out as
   scores^T [k_pos, q_pos]; softmax denominators come from an extra ones
   column appended to v (attn@v then yields z rows 0..63 and den at row 64).
 - exp/normalize: exp on ACT straight out of PSUM, denominators reciprocal'd
   on DVE (1-instr approx, ~51 ULP), broadcast across partitions on GpSimd,
   multiplied during the PSUM->SBUF move on DVE.
All matmuls run in bf16 (1 cyc/row on PE vs 4 for f32).
"""

import sys

sys.path.insert(0, "/opt/trn_rl_repo")

import numpy as np
import ml_dtypes

import concourse.bass as bass  # noqa: F401
import concourse.tile as tile
from concourse import bacc, mybir
from concourse.bass_utils import run_bass_kernel_spmd

BF16 = mybir.dt.bfloat16
F32 = mybir.dt.float32

B, S, D = 2, 2048, 1024
H, DH = 16, 64
NCORE = 8
HPC = 4          # heads per core
KB = D // 128    # 8 k-blocks over the model dim
QC = 512         # q-chunk width
NQC = S // QC    # 4 q-chunks
NKT = S // 128   # 16 k-tiles
ROPE_BASE = 10000.0

_cache = {}


def _build():
    nc = bacc.Bacc("TRN2", target_bir_lowering=False, debug=False, num_devices=NCORE)

    x_d = nc.declare_dram_parameter("x", [128, KB, S], BF16, isOutput=False)
    wqk_d = nc.declare_dram_parameter("wqk", [128, KB, 4, 128], BF16, isOutput=False)
    wv_d = nc.declare_dram_parameter("wv", [128, KB, 256], BF16, isOutput=False)
    wo_d = nc.declare_dram_parameter("wo", [128, 2, 1024], BF16, isOutput=False)
    cos_d = nc.declare_dram_parameter("cos", [128, S], BF16, isOutput=False)
    sin_d = nc.declare_dram_parameter("sin", [128, S], BF16, isOutput=False)
    mask_d = nc.declare_dram_parameter("mask", [128, 2, 128], F32, isOutput=False)
    out_d = nc.declare_dram_parameter("out", [D, S], BF16, isOutput=True)

    Exp = mybir.ActivationFunctionType.Exp

    with tile.TileContext(nc) as tc:
        with (
            tc.tile_pool(name="const", bufs=1) as cp,
            tc.tile_pool(name="ppool", bufs=2, space="PSUM") as ppool,
            tc.tile_pool(name="spool", bufs=2, space="PSUM") as spool,
            tc.tile_pool(name="zpool", bufs=2, space="PSUM") as zpool,
            tc.tile_pool(name="epool", bufs=18) as epool,
            tc.tile_pool(name="work", bufs=2) as work,
            tc.tile_pool(name="nrm", bufs=2) as nrm,
            tc.tile_pool(name="ob", bufs=3) as obp,
        ):
            # ---- load constants ----
            wv = cp.tile([128, KB, 256], BF16, tag="wv")
            nc.scalar.dma_start(wv[:], wv_d[:])
            x_sb = cp.tile([128, KB, S], BF16, tag="x")
            for half in range(2):
                for kb in range(KB):
                    nc.sync.dma_start(
                        x_sb[:, kb, half * 1024:(half + 1) * 1024],
                        x_d[:, kb, half * 1024:(half + 1) * 1024],
                    )
            wqk = cp.tile([128, KB, 4, 128], BF16, tag="wqk")
            nc.sync.dma_start(wqk[:], wqk_d[:])
            wo = cp.tile([128, 2, 1024], BF16, tag="wo")
            nc.sync.dma_start(wo[:], wo_d[:])
            cos_sb = cp.tile([128, S], BF16, tag="cos")
            nc.sync.dma_start(cos_sb[:], cos_d[:])
            sin_sb = cp.tile([128, S], BF16, tag="sin")
            nc.sync.dma_start(sin_sb[:], sin_d[:])
            mask_sb = cp.tile([128, 2, 128], F32, tag="mask")
            nc.scalar.dma_start(mask_sb[:], mask_d[:])

            v_sb = cp.tile([128, NKT, HPC, 65], BF16, tag="v")
            nc.gpsimd.memset(v_sb[:, :, :, 64:65], 1.0)

            # warm the ACT exp table while DMAs stream in
            warm = work.tile([1, 8], F32, tag="warm")
            nc.vector.memset(warm[:], 0.0)
            nc.scalar.activation(warm[:], warm[:], Exp, scale=1.0)

            # ---- v projection: v[s, c] (s on partitions) ----
            for t in range(NKT):
                ps = ppool.tile([128, 512], F32, tag="proj")
                for kb in range(KB):
                    nc.tensor.matmul(
                        ps[:, 0:256],
                        x_sb[:, kb, t * 128:(t + 1) * 128],
                        wv[:, kb, :],
                        start=(kb == 0),
                        stop=(kb == KB - 1),
                    )
                nc.vector.tensor_copy(
                    v_sb[:, t, :, 0:64],
                    ps[:, 0:256].rearrange("p (h d) -> p h d", h=HPC),
                )

            # ---- q/k projection, transposed: [128 = 2 heads x 64, S] ----
            # mt 0,1 = q pairs; mt 2,3 = k pairs. Projection + RoPE run in
            # 1024-wide sequence halves; attention on the first half is
            # emitted before the second half projects, so ACT/DVE attention
            # work overlaps PE projection work.
            T = [cp.tile([128, S], BF16, tag=f"T{mt}", name=f"T{mt}") for mt in range(4)]
            Tpre = [cp.tile([128, S], BF16, tag=f"Tpre{mt}", name=f"Tpre{mt}") for mt in range(4)]
            zsb = [cp.tile([128, S], BF16, tag=f"zsb{p}", name=f"zsb{p}") for p in range(2)]

            def emit_qk_proj_half(nn, mts=(0, 2, 1, 3)):
                lo = nn * 1024
                for mt in mts:
                    for n2 in range(2):
                        n = nn * 2 + n2
                        ps = ppool.tile([128, 512], F32, tag="proj", name=f"qk{mt}_{n}")
                        for kb in range(KB):
                            nc.tensor.matmul(
                                ps[:],
                                wqk[:, kb, mt, :],
                                x_sb[:, kb, n * 512:(n + 1) * 512],
                                start=(kb == 0),
                                stop=(kb == KB - 1),
                            )
                        # Tpre copies stay off DVE: during the interleaved
                        # second half, DVE latency gates zt recycling (norm)
                        nc.scalar.copy(Tpre[mt][:, n * 512:(n + 1) * 512], ps[:])
                    # RoPE on this half (partition-dim rotate-half via DMAs)
                    sw = work.tile([128, 1024], BF16, tag="sw")
                    for g in range(4):
                        src = (g + 1) if g % 2 == 0 else (g - 1)
                        nc.scalar.dma_start(
                            sw[g * 32:(g + 1) * 32, :],
                            Tpre[mt][src * 32:(src + 1) * 32, lo:lo + 1024],
                        )
                    t1 = work.tile([128, 1024], BF16, tag="t1")
                    nc.vector.tensor_mul(t1[:], Tpre[mt][:, lo:lo + 1024], cos_sb[:, lo:lo + 1024])
                    t2 = work.tile([128, 1024], BF16, tag="t2")
                    nc.vector.tensor_mul(t2[:], sw[:], sin_sb[:, lo:lo + 1024])
                    nc.vector.tensor_add(T[mt][:, lo:lo + 1024], t1[:], t2[:])

            # ---- attention (scores^T, exp, attn@v with den row) ----
            emit_qk_proj_half(0)
            # second-half projections interleave at pair granularity: one
            # ~7us PE burst per attention group keeps ACT's 2-deep score
            # pipeline from draining during a single long projection stretch
            proj1_at = {(1, 0): (0,), (1, 1): (2,), (2, 0): (1,), (2, 1): (3,)}
            for j in range(NQC):
                for pair in range(2):
                    if (j, pair) in proj1_at:
                        emit_qk_proj_half(1, mts=proj1_at[(j, pair)])
                    kmax = 4 * (j + 1)
                    iorder = list(range(kmax))
                    exs = {}
                    for i in iorder:
                        qlo = max(0, 128 * i - QC * j)
                        sc = spool.tile([128, 2, QC], F32, tag="sc")
                        for hh in range(2):
                            nc.tensor.matmul(
                                sc[:, hh, qlo:QC],
                                T[2 + pair][hh * 64:(hh + 1) * 64, 128 * i:128 * (i + 1)],
                                T[pair][hh * 64:(hh + 1) * 64, QC * j + qlo:QC * (j + 1)],
                                start=True,
                                stop=True,
                            )
                        if 128 * i >= QC * j:
                            nc.vector.tensor_add(
                                sc[:, :, qlo:qlo + 128],
                                sc[:, :, qlo:qlo + 128],
                                mask_sb[:],
                            )
                        ex = epool.tile([128, 2, QC], BF16, tag="ex", name=f"ex{i}")
                        nc.scalar.activation(
                            ex[:, :, qlo:], sc[:, :, qlo:], Exp, scale=0.125
                        )
                        exs[i] = (ex, qlo)
                    for hh in range(2):
                        zt = zpool.tile([128, QC], F32, tag="zt")
                        for n_i, i in enumerate(iorder):
                            ex, qlo = exs[i]
                            nc.tensor.matmul(
                                zt[0:65, qlo:],
                                v_sb[:, i, pair * 2 + hh, :],
                                ex[:, hh, qlo:],
                                start=(n_i == 0),
                                stop=(n_i == kmax - 1),
                            )
                        # normalize: z / den, den = row 64 of zt
                        # (custom-DVE recip reads PSUM wrong at partition 64 -> copy first)
                        den = nrm.tile([1, QC], F32, tag="den")
                        nc.vector.tensor_copy(den[:], zt[64:65, :])
                        rec = nrm.tile([1, QC], F32, tag="rec")
                        nc.vector.reciprocal_approx_fast(rec[:], den[:])
                        bc = nrm.tile([64, QC], F32, tag="bc")
                        nc.gpsimd.partition_broadcast(bc[:], rec[:])
                        nc.vector.tensor_mul(
                            zsb[pair][hh * 64:(hh + 1) * 64, QC * j:QC * (j + 1)],
                            zt[0:64, :],
                            bc[:],
                        )
                # ---- output projection, pipelined one q-chunk behind ----
                for jj in ([j - 1] if j > 0 else []) + ([j] if j == NQC - 1 else []):
                    _emit_outproj(nc, ppool, obp, wo, zsb, out_d, jj)

    nc.compile()
    return nc


def _emit_outproj(nc, ppool, obp, wo, zsb, out_d, j, pairs=(0, 1), pos=None, done=True):
    for m in range(8):
        po = ppool.tile([128, 512], F32, tag="proj", name=f"po{j}_{m}") if pos is None else pos[m]
        for p in pairs:
            nc.tensor.matmul(
                po[:],
                wo[:, p, m * 128:(m + 1) * 128],
                zsb[p][:, QC * j:QC * (j + 1)],
                start=(p == pairs[0] and pos is None),
                stop=(p == pairs[-1] and done),
            )
        if done:
            ob = obp.tile([128, 512], BF16, tag="ob", name=f"ob{j}_{m}")
            nc.vector.tensor_copy(ob[:], po[:])
            nc.sync.dma_start(out_d[m * 128:(m + 1) * 128, QC * j:QC * (j + 1)], ob[:])


def _rope_tables():
    inv_freq = 1.0 / (ROPE_BASE ** (np.arange(0, DH, 2, dtype=np.float32) / DH))
    t = np.arange(S, dtype=np.float32)
    freqs = np.outer(t, inv_freq)            # [S, 32]
    cosT = np.cos(freqs).T                   # [32, S]
    sinT = np.sin(freqs).T
    cos128 = np.concatenate([cosT, cosT, cosT, cosT], axis=0)
    sin128 = np.concatenate([-sinT, sinT, -sinT, sinT], axis=0)
    return cos128.astype(ml_dtypes.bfloat16), sin128.astype(ml_dtypes.bfloat16)


def _prep_in_maps(x, w_qkv, w_o):
    cos128, sin128 = _rope_tables()
    kp, qc = np.meshgrid(np.arange(128), np.arange(128), indexing="ij")
    mask1 = np.where(kp <= qc, 0.0, -1e9).astype(np.float32)         # [128k, 128q]
    mask = np.ascontiguousarray(np.stack([mask1, mask1], axis=1))    # [128, 2, 128]

    in_maps = []
    for c in range(NCORE):
        b, hb = c // 4, (c % 4) * HPC
        xb = np.ascontiguousarray(x[b].T)                        # [D, S]
        x_sb = xb.reshape(KB, 128, S).transpose(1, 0, 2)         # [128, KB, S]

        wqk = np.empty((128, KB, 4, 128), np.float32)
        for pair in range(2):
            qrows = w_qkv[(hb + 2 * pair) * DH:(hb + 2 * pair + 2) * DH, :]   # [128, D]
            krows = w_qkv[D + (hb + 2 * pair) * DH:D + (hb + 2 * pair + 2) * DH, :]
            wqk[:, :, pair, :] = qrows.T.reshape(KB, 128, 128).transpose(1, 0, 2)
            wqk[:, :, 2 + pair, :] = krows.T.reshape(KB, 128, 128).transpose(1, 0, 2)

        vrows = w_qkv[2 * D + hb * DH:2 * D + (hb + HPC) * DH, :]             # [256, D]
        wv = vrows.T.reshape(KB, 128, 256).transpose(1, 0, 2)                 # [128, KB, 256]

        wo_blk = w_o[:, hb * DH:hb * DH + 256]                                # [1024, 256]
        wo = wo_blk.T.reshape(2, 128, 1024).transpose(1, 0, 2)                # [128, 2, 1024]

        in_maps.append({
            "x": x_sb.astype(ml_dtypes.bfloat16),
            "wqk": wqk.astype(ml_dtypes.bfloat16),
            "wv": wv.astype(ml_dtypes.bfloat16),
            "wo": wo.astype(ml_dtypes.bfloat16),
            "cos": cos128,
            "sin": sin128,
            "mask": mask,
        })
    return in_maps


def get_nc():
    if "nc" not in _cache:
        _cache["nc"] = _build()
    return _cache["nc"]


def run(x, w_qkv, w_o, **runkw):
    nc = get_nc()
    in_maps = _prep_in_maps(np.asarray(x), np.asarray(w_qkv), np.asarray(w_o))
    res = run_bass_kernel_spmd(nc, in_maps, core_ids=list(range(NCORE)), **runkw)
    out = np.zeros((B, S, D), np.float32)
    for c in range(NCORE):
        out[c // 4] += res.results[c]["out"].astype(np.float32).T
    return out, res


def kernel(x, w_qkv, w_o):
    out, _ = run(x, w_qkv, w_o)
    return out
